# revision 57
# baseline (speedup 1.0000x reference)
"""Trainium2 Bass kernel for nn_DGCLoss (DCG/NDCG ranking loss).

v4 structure.  For row n with s = cosine-sim row mapped to [0,1] (diag
masked to -1e6):
    u(i) = indicator+1 = sum_j sigma(1000*(s_j - s_i)) + 1.5
    dcg[n] = sum_i rel0[n,i] * ln2 / ln(u(i))

Whole rows are assigned to one of two lanes (period-4 [D,D,D,A]):
  A-rows (12, ACT lane, exact sigmoid):
    PE broadcasts the s-row across partitions via TWO bf16 matmuls
    (hi + lo split of s, PSUM fp32 accumulate: error ~2e-6 -- plain fp32
    matmuls are 4 cyc/row, and fp32r hardware rounding is bf16-coarse,
    which broke accuracy), then 3 ACT sigmoid chunks with fused
    accum_out row-sums.
  D-rows (36, DVE lane, step-function sigmoid, validated end-to-end):
    gpsimd partition_broadcast replicates THREE consecutive rows from a
    partition-0 flat copy of s into SBUF per instruction (Pool hardware
    is SBUF-only and its source must sit on partition 0), then 3 DVE
    tensor_scalar is_gt chunks with accum_out in the 2x_2P mode
    (single-src fp32 SBUF).  The j==i tie's +0.5 rides the tail Ln bias
    (hardware applies the TS second scalar once, not per element).

Antisymmetric evaluation: chunk c (i in [128c,128c+128)) only computes
j in [0, 128(c+1)); the missing upper blocks use sigma(z)+sigma(-z)=1
(1[a>b] = 1-1[b>a]): their column sums come from nearly-free PE matmuls
with the junk tile as stationary, subtracted in the tail, with the block
constants (256/128/0) folded into per-c Ln biases.

idcg depends only on gt (host); normalize/relevance prep on host.
"""

import math

import numpy as np

N = 384
D = 256
NCORES = 8
RPC = N // NCORES  # 48 rows per core
NCHUNK = 3 * RPC   # 144 chunks per core
EPS = 1e-8
NEG_BIG = -1.0e6
LN2 = math.log(2.0)
BETA = 1000.0      # sigmoid sharpness on the [0,1]-mapped similarity scale

_CACHE = {}


def _row_assign():
    """Row -> lane, exact period-4 [D,D,D,A]: 12 A / 36 D rows.  Balances
    ACT row cost (~1756 ns) against DVE (581) + Pool broadcast; the
    regular pattern keeps each period's three D-rows contiguous so one
    gpsimd partition_broadcast serves all three."""
    return ["A" if r % 4 == 3 else "D" for r in range(RPC)]


_ROW_ENG = _row_assign()


def _assign(k):
    """Engine for chunk k (k = 3*r + c): the row's engine."""
    return _ROW_ENG[k // 3]


def _assignment():
    """Per-chunk (engine, slot-within-engine); plus per-engine totals."""
    engs, slots, counts = [], [], {"A": 0, "D": 0}
    for k in range(NCHUNK):
        e = _assign(k)
        engs.append(e)
        slots.append(counts[e])
        counts[e] += 1
    return engs, slots, counts


def _chunk_cols():
    """Column of each chunk k in the engine-blocked [A|D] tail layout."""
    engs, slots, counts = _assignment()
    base = {"A": 0, "D": counts["A"]}
    return [base[engs[k]] + slots[k] for k in range(NCHUNK)]


# ---------------------------------------------------------------- device code


def _build_nc():
    """Build + compile the (SPMD, per-core) Bass program."""
    from contextlib import ExitStack

    import concourse.bacc as bacc
    import concourse.mybir as mybir
    import concourse.tile as tile

    f32 = mybir.dt.float32
    bf16 = mybir.dt.bfloat16
    AF = mybir.ActivationFunctionType
    ALU = mybir.AluOpType

    nc = bacc.Bacc(
        "TRN2",
        target_bir_lowering=False,
        debug=False,
        enable_asserts=True,
        num_devices=NCORES,
    )

    engs, slots, counts = _assignment()
    nA, nD = counts["A"], counts["D"]
    nAr, nDr = nA // 3, nD // 3

    # inputs merged into blobs to cut serialized DMA dispatches:
    #   xnt [128, 2*(768+96)] bf16 = hi|lo split of (xn^T packed | xst)
    #     (the gram runs as bf16 hi/lo-split matmuls: 1 cyc/row and exact
    #      to ~2e-6, vs fp32's 4 cyc/row)
    #   xr  [128, 144]    = relt (engine-blocked layout)
    #   dm  [48, 384+48]  = dmask | i48
    W = 2 * N + 2 * RPC
    xnt_d = nc.dram_tensor("xnt", [128, 2 * W], bf16, kind="ExternalInput")
    xr_d = nc.dram_tensor("xr", [128, NCHUNK], f32, kind="ExternalInput")
    dm_d = nc.dram_tensor("dm", [RPC, N + RPC], f32, kind="ExternalInput")
    dcg_d = nc.dram_tensor("dcg", [1, NCHUNK], f32, kind="ExternalOutput")

    with tile.TileContext(nc) as tc, ExitStack() as ctx:
        const = ctx.enter_context(tc.tile_pool(name="const", bufs=1))
        junka = ctx.enter_context(tc.tile_pool(name="junka", bufs=6))
        junkd = ctx.enter_context(tc.tile_pool(name="junkd", bufs=8))

        # three input DMAs on three different DGE queues so the transfers
        # overlap instead of serializing behind one dispatcher
        xnt_sb = const.tile([128, 2 * W], bf16, name="xnt_sb", tag="xnt")
        # hi half first: the gram's hi*hi terms only need this half, so
        # the PE can start ~0.6us earlier; lo half follows on the queue
        nc.sync.dma_start(xnt_sb[:, 0:W], xnt_d.ap()[:, 0:W])
        nc.sync.dma_start(xnt_sb[:, W : 2 * W], xnt_d.ap()[:, W : 2 * W])
        xr_sb = const.tile([128, NCHUNK], f32, name="xr_sb", tag="xr")
        nc.gpsimd.dma_start(xr_sb[:], xr_d.ap()[:])
        dm_sb = const.tile([RPC, N + RPC], f32, name="dm_sb", tag="dm")
        nc.scalar.dma_start(dm_sb[:], dm_d.ap()[:])

        # dummy sigmoid right after ACT's dma dispatch: hoists the ACT
        # table load off the critical path into the input-DMA window
        warm = const.tile([1, 1], f32, name="warm", tag="warm")
        nc.vector.memset(warm[:], 1.0)
        nc.scalar.activation(warm[:], warm[:], AF.Sigmoid)

        xh = xnt_sb[:, 0:W]
        xl = xnt_sb[:, W : 2 * W]
        relt_sb = xr_sb[:, 0:NCHUNK]
        dmask_sb = dm_sb[0:RPC, 0:N]
        i48_sb = dm_sb[0:RPC, N : N + RPC]

        ones_sb = const.tile([128, 1], f32, name="ones_sb", tag="ones")
        nc.vector.memset(ones_sb[:], 1.0)
        mones = const.tile([128, 1], f32, name="mones", tag="mones")
        nc.vector.memset(mones[:], 1.0)

        s_sb = const.tile([RPC, N], f32, name="s_sb", tag="s")
        # flat copy of s~ on partition 0: gpsimd partition_broadcast can
        # only source from partition 0
        s_flat = const.tile([1, RPC * N], f32, name="s_flat", tag="s_flat")
        # bf16 hi/lo split of s~ for the exact-to-2e-6 A-row broadcasts
        shi = const.tile([RPC, N], bf16, name="shi", tag="shi")
        slo = const.tile([RPC, N], bf16, name="slo", tag="slo")
        i48b = const.tile([RPC, RPC], bf16, name="i48b", tag="i48b")
        bneg = [
            const.tile([128, RPC], f32, name=f"bneg{c}", tag=f"bneg{c}")
            for c in range(3)
        ]
        pposD = [
            const.tile([128, RPC], f32, name=f"pposD{c}", tag=f"pposD{c}")
            for c in range(3)
        ]
        # per-engine accumulators: single-writer tiles
        acc = {
            "A": const.tile([128, nA], f32, name="accA", tag="accA"),
            "D": const.tile([128, nD], f32, name="accD", tag="accD"),
        }

        # ---- phase 1: gram slice (fp32), diag mask, transposed threshold
        # tiles (fp32), hi/lo split
        with tc.tile_pool(name="pg", bufs=1, space="PSUM") as pgp, tc.tile_pool(
            name="pt", bufs=1, space="PSUM"
        ) as ptp:
            pg = pgp.tile([RPC, N], f32, name="pg", tag="pg")
            hi_mms, lo_mms = [], []
            for h in range(2):  # K-chunk (d 0:128 / 128:256)
                xs_h = xh[:, 2 * N + RPC * h : 2 * N + RPC * (h + 1)]
                xs_l = xl[:, 2 * N + RPC * h : 2 * N + RPC * (h + 1)]
                xn_h = xh[:, N * h : N * (h + 1)]
                xn_l = xl[:, N * h : N * (h + 1)]
                hi_mms.append((xs_h, xn_h))
                lo_mms += [(xs_h, xn_l), (xs_l, xn_h)]
            mms = hi_mms + lo_mms
            for mi, (a, b) in enumerate(mms):
                nc.tensor.matmul(
                    pg[:], a, b, start=(mi == 0), stop=(mi == len(mms) - 1)
                )
            # s~ = cosine row block + diagonal mask (-1e6 at global diag)
            nc.vector.tensor_add(s_sb[:], pg[:], dmask_sb[:])
            nc.sync.dma_start(s_flat[0:1, 0 : 3 * N], s_sb[0:3, :])
            nc.sync.dma_start(
                s_flat[0:1, 3 * N : RPC * N], s_sb[3:RPC, :]
            )
            nc.vector.tensor_copy(shi[:], s_sb[:])
            nc.vector.tensor_sub(slo[:], s_sb[:], shi[:])
            nc.vector.tensor_copy(i48b[:], i48_sb[:])
            for c in range(3):
                pt = ptp.tile([128, RPC], f32, name=f"pt{c}", tag=f"pt{c}")
                nc.tensor.transpose(
                    pt[:], s_sb[0:RPC, 128 * c : 128 * (c + 1)], i48_sb[:]
                )
                # ACT bias = -BETA * s_i^T ; DVE threshold = +s_i^T
                nc.vector.tensor_scalar_mul(bneg[c][:], pt[:], -BETA)
                nc.scalar.mul(pposD[c][:], pt[:], 1.0)

        # ---- phase 2: main loop (see module docstring)
        last_act = None
        cs = {}
        pend = []
        csp = ctx.enter_context(tc.tile_pool(name="cs", bufs=1, space="PSUM"))
        cs_all = csp.tile([128, 2 * nAr + 2 * nDr], f32, name="cs", tag="cs")
        cs["A"] = cs_all[:, 0 : 2 * nAr]
        cs["D"] = cs_all[:, 2 * nAr : 2 * nAr + 2 * nDr]

        def _emit_colsums(eng, rowi, junks):
            cse = cs[eng]
            for tc2 in range(2):
                col = cse[:, 2 * rowi + tc2 : 2 * rowi + tc2 + 1]
                srcs = list(range(tc2 + 1, 3))
                for si, c in enumerate(srcs):
                    nc.tensor.matmul(
                        col,
                        junks[c][:, 128 * tc2 : 128 * (tc2 + 1)],
                        mones[:],
                        start=(si == 0),
                        stop=(si == len(srcs) - 1),
                    )

        # D-rows: gpsimd broadcasts THREE consecutive D-rows (4p..4p+2,
        # contiguous in s_flat) per instruction to amortize the Q7 launch
        # and cut sync traffic 3x
        d_group = {}  # row -> (group index, position, rows)
        for r in range(RPC):
            if engs[3 * r] == "D":
                g0 = 4 * (r // 4)
                ng = 4 if engs[3 * (min(g0 + 3, RPC - 1))] == "D" else 3
                d_group[r] = (r // 4, r % 4, list(range(g0, g0 + ng)))
        pbg_tiles = {}
        with tc.tile_pool(
            name="pbA", bufs=5, space="PSUM"
        ) as pbap, tc.tile_pool(name="pbg", bufs=4) as pbgp:
            for r in range(RPC):
                eng = engs[3 * r]
                rowi = slots[3 * r] // 3
                if eng == "A":
                    pb = pbap.tile([128, N], f32, name=f"pb{r}", tag="pbA")
                    onehot = i48b[:, r : r + 1].broadcast_to([RPC, 128])
                    nc.tensor.matmul(
                        pb[:], onehot, shi[:], start=True, stop=False
                    )
                    nc.tensor.matmul(
                        pb[:], onehot, slo[:], start=False, stop=True
                    )
                else:
                    gidx, pos, grp = d_group[r]
                    if gidx not in pbg_tiles:
                        gt_ = pbgp.tile(
                            [128, N * len(grp)],
                            f32,
                            name=f"pbg{gidx}",
                            tag="pbg",
                        )
                        nc.gpsimd.partition_broadcast(
                            gt_[:],
                            s_flat[0:1, N * grp[0] : N * (grp[0] + len(grp))],
                        )
                        pbg_tiles[gidx] = gt_
                    pb = pbg_tiles[gidx][:, N * pos : N * (pos + 1)]
                if len(pend) >= 6:
                    _emit_colsums(*pend.pop(0))
                junks = []
                for c in range(3):
                    k = 3 * r + c
                    jr = 128 * (c + 1)
                    a_col = acc[eng][:, slots[k] : slots[k] + 1]
                    if eng == "A":
                        ja = junka.tile([128, N], f32, name=f"ja{k}", tag="ja")
                        last_act = nc.scalar.activation(
                            ja[:, 0:jr],
                            pb[:, 0:jr],
                            AF.Sigmoid,
                            bias=bneg[c][:, r : r + 1],
                            scale=BETA,
                            accum_out=a_col,
                        )
                        junks.append(ja)
                    else:
                        jd = junkd.tile([128, N], f32, name=f"jd{k}", tag="jd")
                        # no per-element tie constant: hw applies op1's
                        # scalar2 once, not per element; the +0.5 tie
                        # rides the per-lane tail Ln bias instead
                        nc.vector.tensor_scalar(
                            out=jd[:, 0:jr],
                            in0=pb[:, 0:jr],
                            scalar1=pposD[c][:, r : r + 1],
                            scalar2=0.0,
                            op0=ALU.is_gt,
                            op1=ALU.add,
                            accum_out=a_col,
                        )
                        junks.append(jd)
                pend.append((eng, rowi, junks))
            for job in pend:
                _emit_colsums(*job)

        # dummy ln ordered right after the final sigmoid chunk: the ~1.3us
        # ln-set table load overlaps the remaining DVE chunks
        warm_ln = nc.scalar.activation(warm[:], warm[:], AF.Ln)
        tile.add_dep_helper(
            warm_ln.ins,
            last_act.ins,
            reason="hoist ln table load right after final sigmoid",
        )

        # ---- phase 3: tail.  u = acc - cs + bias_c, bias_c = 1.5 +
        # 256/128/0 mirror-block constants; per-chunk dcg = ones^T @
        # (relt / ln(u)); the host sums each row's 3 entries.
        with tc.tile_pool(name="pd", bufs=1, space="PSUM") as pdp:
            pd = pdp.tile([1, NCHUNK], f32, name="pd", tag="pd")
            lnu = const.tile([128, NCHUNK], f32, name="lnu", tag="lnu")
            dterm = const.tile([128, NCHUNK], f32, name="dterm", tag="dterm")
            # per-(lane, c) Ln bias: 1.5 + mirror-block constant
            # (256/128/0) + the j==i tie 0.5 for the step lane (its is_gt
            # contributes 0 there; sigma contributes it directly)
            ub = {}
            for e, tie in (("A", 0.0), ("D", 0.5)):
                for c, extra in ((0, 256.0), (1, 128.0), (2, 0.0)):
                    t = const.tile(
                        [128, 1], f32, name=f"ub{e}{c}", tag=f"ub{e}{c}"
                    )
                    nc.vector.memset(t[:], 1.5 + tie + extra)
                    ub[(e, c)] = t
            off = 0
            for e, n_e in (("A", nA), ("D", nD)):
                av = acc[e][:].rearrange("p (r c) -> p r c", c=3)
                # subtract the mirror colsums from the c=0,1 slots
                nc.vector.tensor_sub(
                    av[:, :, 0:2],
                    av[:, :, 0:2],
                    cs[e].rearrange("p (r c) -> p r c", c=2),
                )
                for c in range(3):
                    ln_inst = nc.scalar.activation(
                        lnu[:, off : off + n_e].rearrange(
                            "p (r c) -> p r c", c=3
                        )[:, :, c],
                        av[:, :, c],
                        AF.Ln,
                        bias=ub[(e, c)][:],
                        scale=1.0,
                    )
                    tile.add_dep_helper(
                        ln_inst.ins,
                        last_act.ins,
                        reason="batch ACT table sets: all sigmoid before ln",
                    )
                off += n_e
            nc.vector.reciprocal(lnu[:, 0:nA], lnu[:, 0:nA])
            nc.vector.tensor_mul(
                dterm[:, 0:nA], lnu[:, 0:nA], relt_sb[:, 0:nA]
            )
            nc.tensor.matmul(
                pd[:, 0:nA], ones_sb[:], dterm[:, 0:nA],
                start=True, stop=True,
            )
            nc.vector.reciprocal(lnu[:, nA:NCHUNK], lnu[:, nA:NCHUNK])
            nc.vector.tensor_mul(
                dterm[:, nA:NCHUNK], lnu[:, nA:NCHUNK], relt_sb[:, nA:NCHUNK]
            )
            nc.tensor.matmul(
                pd[:, nA:NCHUNK], ones_sb[:], dterm[:, nA:NCHUNK],
                start=True, stop=True,
            )
            out_sb = const.tile([1, NCHUNK], f32, name="out_sb", tag="out")
            nc.vector.tensor_copy(out_sb[:], pd[:])
            nc.sync.dma_start(dcg_d.ap()[:], out_sb[:])

    nc.compile()
    return nc


def _get_nc():
    if "nc" not in _CACHE:
        _CACHE["nc"] = _build_nc()
    return _CACHE["nc"]


# ------------------------------------------------------------------ execution


def _get_runner():
    """Cached jitted 8-core SPMD executor (modeled on bass2jax's
    run_bass_via_pjrt multi-core path, but reusable across calls)."""
    if "runner" in _CACHE:
        return _CACHE["runner"]

    import jax
    from jax.sharding import Mesh, PartitionSpec
    from jax.experimental.shard_map import shard_map

    import concourse.mybir as mybir
    from concourse.bass2jax import (
        _bass_exec_p,
        install_neuronx_cc_hook,
        partition_id_tensor,
    )

    nc = _get_nc()
    install_neuronx_cc_hook()

    partition_name = (
        nc.partition_id_tensor.name if nc.partition_id_tensor else None
    )
    in_names, out_names, out_avals, zero_outs = [], [], [], []
    for alloc in nc.m.functions[0].allocations:
        if not isinstance(alloc, mybir.MemoryLocationSet):
            continue
        name = alloc.memorylocations[0].name
        if alloc.kind == "ExternalInput":
            if name != partition_name:
                in_names.append(name)
        elif alloc.kind == "ExternalOutput":
            shape = tuple(alloc.tensor_shape)
            dtype = mybir.dt.np(alloc.dtype)
            out_avals.append(jax.core.ShapedArray(shape, dtype))
            out_names.append(name)
            zero_outs.append(np.zeros(shape, dtype))
    n_params = len(in_names)
    n_outs = len(out_avals)
    all_in_names = in_names + out_names
    if partition_name is not None:
        all_in_names = all_in_names + [partition_name]

    def _body(*args):
        operands = list(args)
        if partition_name is not None:
            operands.append(partition_id_tensor())
        outs = _bass_exec_p.bind(
            *operands,
            out_avals=tuple(out_avals),
            in_names=tuple(all_in_names),
            out_names=tuple(out_names),
            lowering_input_output_aliases=(),
            sim_require_finite=True,
            sim_require_nnan=True,
            nc=nc,
        )
        return tuple(outs)

    devices = jax.devices()[:NCORES]
    assert len(devices) == NCORES, f"need {NCORES} cores, got {len(devices)}"
    mesh = Mesh(np.asarray(devices), ("core",))
    in_specs = (PartitionSpec("core"),) * (n_params + n_outs)
    out_specs = (PartitionSpec("core"),) * n_outs
    sharded = jax.jit(
        shard_map(
            _body, mesh=mesh, in_specs=in_specs, out_specs=out_specs,
            check_rep=False,
        ),
        keep_unused=True,
    )

    def make_args(in_maps, on_device=False):
        concat_in = [
            np.concatenate([np.asarray(m[name]) for m in in_maps], axis=0)
            for name in in_names
        ]
        concat_zeros = [
            np.zeros((NCORES * z.shape[0], *z.shape[1:]), z.dtype)
            for z in zero_outs
        ]
        args = concat_in + concat_zeros
        if on_device:
            from jax.sharding import NamedSharding

            sh = NamedSharding(mesh, PartitionSpec("core"))
            args = [jax.device_put(a, sh) for a in args]
            jax.block_until_ready(args)
        return args

    def unpack(out_arrs):
        return [
            {
                name: np.asarray(out_arrs[i]).reshape(
                    NCORES, *out_avals[i].shape
                )[c]
                for i, name in enumerate(out_names)
            }
            for c in range(NCORES)
        ]

    def run(in_maps, blocking=True):
        out_arrs = sharded(*make_args(in_maps))
        if not blocking:
            return out_arrs
        return unpack(out_arrs)

    run.sharded = sharded
    run.make_args = make_args
    run.unpack = unpack
    _CACHE["runner"] = run
    return run


# ---------------------------------------------------------------- host logic


def _prepare_in_maps(ranking, gt):
    x = np.asarray(ranking, dtype=np.float32)
    gtv = np.asarray(gt).astype(np.int64)
    assert x.shape == (N, D), x.shape

    norms = np.linalg.norm(x, axis=1, keepdims=True).astype(np.float32)
    xn = (x / np.clip(norms, EPS, None)).astype(np.float32)
    xnT = xn.T  # [D, N]
    xnt = np.ascontiguousarray(
        np.concatenate([xnT[0:128], xnT[128:256]], axis=1)
    )  # [128, 2N]

    g = np.abs(gtv[None, :] - gtv[:, None]).astype(np.float32)
    rel = (np.exp2(np.clip(10.0 - g, 0.0, None)) - 1.0).astype(np.float32)
    rel[np.arange(N), np.arange(N)] = 0.0

    i48 = np.eye(RPC, dtype=np.float32)
    cols = _chunk_cols()

    import ml_dtypes

    bf16 = ml_dtypes.bfloat16
    in_maps = []
    for c in range(NCORES):
        n0 = c * RPC
        xsT = xn[n0 : n0 + RPC].T  # [D, RPC]
        xst = np.concatenate([xsT[0:128], xsT[128:256]], axis=1)  # [128, 96]
        dmask = np.zeros((RPC, N), dtype=np.float32)
        dmask[np.arange(RPC), n0 + np.arange(RPC)] = NEG_BIG
        rs = rel[n0 : n0 + RPC] * np.float32(LN2)  # [RPC, N]
        # relt[p, col(k)] = rs[r, 128c+p] for chunk k=3r+c, where col(k)
        # is the chunk's column in the engine-blocked [A|D] layout
        relt = np.zeros((128, NCHUNK), dtype=np.float32)
        for k in range(NCHUNK):
            r, cc = divmod(k, 3)
            relt[:, cols[k]] = rs[r, 128 * cc : 128 * (cc + 1)]
        xfull = np.concatenate([xnt, xst], axis=1)  # [128, 864] fp32
        xhi = xfull.astype(bf16)
        xlo = (xfull - xhi.astype(np.float32)).astype(bf16)
        in_maps.append(
            {
                "xnt": np.ascontiguousarray(
                    np.concatenate([xhi, xlo], axis=1)
                ),
                "xr": np.ascontiguousarray(relt),
                "dm": np.ascontiguousarray(
                    np.concatenate([dmask, i48], axis=1)
                ),
            }
        )
    return in_maps, gtv


def _idcg_per_row(gtv):
    """idcg depends only on gt[n]; reproduce the reference's sorted-rel sum."""
    M = N - 1
    disc = np.log2(np.arange(M, dtype=np.float32) + 2.0).astype(np.float32)
    gtv = gtv - gtv.min()  # |gt_i - gt_j| is shift-invariant; bincount needs >= 0
    maxv = int(gtv.max())
    hist = np.bincount(gtv, minlength=maxv + 1)
    idcg_by_val = {}
    for a in np.unique(gtv):
        a = int(a)
        chunks = []
        d = 0
        while True:
            if d == 0:
                cnt = hist[a] - 1
            else:
                cnt = 0
                if a - d >= 0:
                    cnt += hist[a - d]
                if a + d <= maxv:
                    cnt += hist[a + d]
                if a - d < 0 and a + d > maxv:
                    break
            v = np.float32(2.0 ** max(10.0 - d, 0.0) - 1.0)
            chunks.append(np.full(cnt, v, dtype=np.float32))
            d += 1
        rel_sorted = np.concatenate(chunks)
        assert rel_sorted.shape == (M,)
        idcg_by_val[a] = np.float32(
            np.sum((rel_sorted / disc).astype(np.float32), dtype=np.float32)
        )
    return np.array([idcg_by_val[int(a)] for a in gtv], dtype=np.float32)


def _finalize(dcg, gtv):
    idcg = _idcg_per_row(gtv)
    valid = idcg != 0.0
    ndcg = np.where(
        valid, dcg / np.where(valid, idcg, np.float32(1.0)), np.float32(0.0)
    ).astype(np.float32)
    cnt = int(valid.sum())
    if cnt == 0:
        return np.float32(1.0)
    mean = np.float32(ndcg.sum(dtype=np.float32) / np.float32(max(cnt, 1)))
    return np.float32(np.float32(1.0) - mean)


def _dcg_from_out(out_flat):
    """Per-core [1, NCHUNK] per-chunk dcg values -> per-row dcg [RPC]."""
    cols = np.asarray(_chunk_cols())
    per_chunk = np.asarray(out_flat, dtype=np.float32).reshape(-1)[cols]
    return per_chunk.reshape(RPC, 3).sum(axis=1, dtype=np.float32)


def kernel(ranking, gt):
    in_maps, gtv = _prepare_in_maps(ranking, gt)
    run = _get_runner()
    results = run(in_maps)
    dcg = np.concatenate(
        [_dcg_from_out(results[c]["dcg"]) for c in range(NCORES)]
    ).astype(np.float32)
    return _finalize(dcg, gtv)
